# revision 23
# baseline (speedup 1.0000x reference)
"""Trainium2 Bass kernel for DepthwiseXCorrAug.

Computes, for B=64 samples sharded 8-per-core across 8 NeuronCores:
  k = relu(bn(conv3x3_valid(kernel_in, w_k)))     # [B,256,5,5]
  s = relu(bn(conv3x3_same(search_in, w_s)))      # [B,256,31,31]
  out = per-sample per-channel xcorr(s, k), pad 2 # [B,256,31,31]

Device strategy (per core):
  - all data-path tensors in bf16 (halves DMA bytes; PE rate is identical
    to fp32r); BN folded into weights on host, bias+ReLU applied by
    ScalarE on PSUM eviction.
  - conv branches as bf16 matmuls over (ci-block x 3x3-tap) accumulated
    in PSUM, streaming exactly 31-wide output windows.
  - depthwise xcorr as bf16 diagonal-weight matmuls: 16 concurrent 32x32
    PE tiles (4 channel-blocks x 4 samples) accumulate the 25 taps.
  - weights packed ob-major and DMA'd critical-first so the first conv
    matmul issues ~4us in; output DMA triggered from the ScalarE HWDGE
    queue so the SP sequencer never stalls the PE between xcorr chunks.
"""

import sys

sys.path.insert(0, "/opt/trn_rl_repo")

import numpy as np

import concourse.bass as bass
import concourse.mybir as mybir
import concourse.tile as tile
from concourse import bacc
from concourse.bass_utils import run_bass_kernel_spmd

EPS = 1e-5
N_CORES = 8
B, CIN, HID = 64, 256, 256
SPC = B // N_CORES  # samples per core

_cached_nc = None
last_results = None  # set by kernel(); used by test harness for profiling


def _build_program():
    f32 = mybir.dt.float32
    bf16 = mybir.dt.bfloat16
    RELU = mybir.ActivationFunctionType.Relu

    nc = bacc.Bacc("TRN2", target_bir_lowering=False, debug=False,
                   num_devices=N_CORES)

    wTs_d = nc.dram_tensor("wTs", [2, 128, 2304], bf16, kind="ExternalInput").ap()
    wTk_d = nc.dram_tensor("wTk", [2, 128, 2304], bf16, kind="ExternalInput").ap()
    xk_d = nc.dram_tensor("xk", [2, 128, 1800], bf16, kind="ExternalInput").ap()
    xs_d = nc.dram_tensor("xs", [SPC, 2, 128, 33 * 34], bf16, kind="ExternalInput").ap()
    bias_d = nc.dram_tensor("bias", [4, 128, 1], f32, kind="ExternalInput").ap()
    m32_d = nc.dram_tensor("m32", [128, 32], bf16, kind="ExternalInput").ap()
    out_d = nc.dram_tensor("out", [SPC, CIN, 31, 31], bf16, kind="ExternalOutput").ap()
    out_flat = out_d.rearrange("s c h w -> s c (h w)")

    with tile.TileContext(nc) as tc:
        with tc.tile_pool(name="wp", bufs=1) as wp, \
             tc.tile_pool(name="spin", bufs=3) as spin_pool, \
             tc.tile_pool(name="spoutp", bufs=1) as spout_pool, \
             tc.tile_pool(name="stripp", bufs=1) as strip_pool, \
             tc.tile_pool(name="xop", bufs=12) as xout_pool, \
             tc.tile_pool(name="psc", bufs=4, space="PSUM") as psc, \
             tc.tile_pool(name="psx", bufs=4, space="PSUM") as psx_pool:

            # ---- persistent inputs (fused tiles; one DMA trigger each) ----
            wTs2 = wp.tile([128, 2 * 2304], bf16, tag="wTs2", name="wTs2")
            wTk2 = wp.tile([128, 2 * 2304], bf16, tag="wTk2", name="wTk2")
            xk2 = wp.tile([128, 2 * 1800], bf16, tag="xk2", name="xk2")
            wTs = [wTs2[:, cb * 2304:(cb + 1) * 2304] for cb in range(2)]
            wTk = [wTk2[:, cb * 2304:(cb + 1) * 2304] for cb in range(2)]
            xk = [xk2[:, cb * 1800:(cb + 1) * 1800] for cb in range(2)]
            bias = wp.tile([128, 4], f32, tag="bias", name="bias")
            bk = [bias[:, ob:ob + 1] for ob in range(2)]
            bs = [bias[:, 2 + ob:3 + ob] for ob in range(2)]
            m32 = wp.tile([128, 32], bf16, tag="m32", name="m32")
            kf = [wp.tile([128, 200], f32, tag=f"kf{ob}", name=f"kf{ob}")
                  for ob in range(2)]

            # spin prefetch state: pairs load as one fused DMA each
            spin_views = {}
            sp_tiles = {}

            def prefetch_pair(pair):
                t_in = spin_pool.tile([128, 4 * 1122], bf16,
                                      tag="spin", name=f"spin{pair}")
                sp_tiles[pair] = t_in
                nc.sync.dma_start(
                    t_in[:].rearrange("p (s c x) -> p s c x", s=2, c=2),
                    xs_d[pair * 2:pair * 2 + 2].rearrange("s c p x -> p s c x"))
                for s in (pair * 2, pair * 2 + 1):
                    for cb in range(2):
                        spin_views[(s, cb)] = t_in[
                            :, ((s % 2) * 2 + cb) * 1122:
                            ((s % 2) * 2 + cb + 1) * 1122].rearrange(
                            "p (h w) -> p h w", h=33, w=34)

            # --- DMA issue order: critical path first; 13 triggers total ---
            sp0 = wp.tile([128, 4 * 1122], bf16, tag="sp0", name="sp0")
            for s in (0, 1):
                for cb in range(2):
                    spin_views[(s, cb)] = sp0[:, (s * 2 + cb) * 1122:
                                              (s * 2 + cb + 1) * 1122].rearrange(
                        "p (h w) -> p h w", h=33, w=34)
            # the first 9 conv matmuls need only (s0, cb0) + the cb0/ob0 taps
            nc.sync.dma_start(sp0[:, 0:1122], xs_d[0, 0])
            nc.sync.dma_start(
                wTs2[:].rearrange("p (c x) -> p c x", c=2)[:, :, 0:1152],
                wTs_d[:, :, 0:1152].rearrange("c p x -> p c x"))
            nc.sync.dma_start(sp0[:, 1122:2244], xs_d[0, 1])
            nc.sync.dma_start(
                sp0[:, 2244:4488].rearrange("p (c x) -> p c x", c=2),
                xs_d[1].rearrange("c p x -> p c x"))
            nc.sync.dma_start(
                xk2[:].rearrange("p (c x) -> p c x", c=2),
                xk_d.rearrange("c p x -> p c x"))
            nc.sync.dma_start(
                wTk2[:].rearrange("p (c x) -> p c x", c=2)[:, :, 0:1152],
                wTk_d[:, :, 0:1152].rearrange("c p x -> p c x"))
            nc.sync.dma_start(bias[:].rearrange("p (b x) -> p b x", b=4),
                              bias_d.rearrange("b p x -> p b x"))
            nc.sync.dma_start(m32[:], m32_d)
            nc.sync.dma_start(
                wTs2[:].rearrange("p (c x) -> p c x", c=2)[:, :, 1152:2304],
                wTs_d[:, :, 1152:2304].rearrange("c p x -> p c x"))
            nc.sync.dma_start(
                wTk2[:].rearrange("p (c x) -> p c x", c=2)[:, :, 1152:2304],
                wTk_d[:, :, 1152:2304].rearrange("c p x -> p c x"))
            prefetch_pair(1)

            # weight column layout is ob-major: col = ob*1152 + t*128
            def w_lhs(w, cb, t, ob):
                c0 = ob * 1152 + t * 128
                return w[cb][:, c0:c0 + 128]

            # ---- conv_k: all 8 samples batched on the free dim (N=200) ----
            def emit_conv_k():
                for ob in range(2):
                    pk = psc.tile([128, 200], f32, tag="conv", name=f"pk{ob}")
                    idx = 0
                    for cb in range(2):
                        for t in range(9):
                            nc.tensor.matmul(
                                pk[:],
                                w_lhs(wTk, cb, t, ob),
                                xk[cb][:, t * 200:(t + 1) * 200],
                                start=(idx == 0), stop=(idx == 17))
                            idx += 1
                    nc.scalar.activation(kf[ob][:], pk[:], RELU,
                                         bias=bk[ob][:, 0:1], scale=1.0)

            # ---- strips: bf16 diagonal weights for the xcorr ----
            strips = {}
            for s in range(SPC):
                for ob in range(2):
                    strips[(s, ob)] = strip_pool.tile(
                        [128, 800], bf16,
                        tag=f"strip{s}_{ob}", name=f"strip{s}_{ob}")

            def emit_strips():
                # st[p, t, c] = m32[p, c] * kf[p, s*25+t] in one broadcasted
                # DVE op per (s, ob): 16 ops instead of 400.
                for ob in range(2):
                    for s in range(SPC):
                        st = strips[(s, ob)]
                        st_v = st[:].rearrange("p (t c) -> p t c", t=25)
                        m32_b = m32[:].unsqueeze(1).broadcast_to((128, 25, 32))
                        kf_b = kf[ob][:, s * 25:(s + 1) * 25].unsqueeze(
                            2).broadcast_to((128, 25, 32))
                        nc.vector.tensor_tensor(st_v, m32_b, kf_b,
                                                mybir.AluOpType.mult)

            # ---- spout tiles (bf16, zero borders) ----
            spout = {}
            for s in range(SPC):
                for ob in range(2):
                    sp = spout_pool.tile([128, 35 * 35], bf16,
                                         tag=f"spout{s}_{ob}", name=f"spout{s}_{ob}")
                    spout[(s, ob)] = sp
                    nc.gpsimd.memset(sp[:], 0.0)

            # ---- main: conv groups interleaved with xcorr chunks ----
            def conv_s_group(samples):
                views = spin_views
                for ob in range(2):
                    ptiles = {}
                    for s in samples:
                        for ci, (y0, nr) in enumerate([(0, 16), (16, 15)]):
                            ptiles[(s, ci)] = psc.tile(
                                [128, nr * 31], f32, tag="conv",
                                name=f"pc{s}_{ob}_{ci}")
                    idx = 0
                    for cb in range(2):
                        for t in range(9):
                            dy, dx = t // 3, t % 3
                            lhsT = w_lhs(wTs, cb, t, ob)
                            for s in samples:
                                for ci, (y0, nr) in enumerate([(0, 16), (16, 15)]):
                                    nc.tensor.matmul(
                                        ptiles[(s, ci)][:],
                                        lhsT,
                                        views[(s, cb)][:, y0 + dy:y0 + dy + nr,
                                                       dx:dx + 31],
                                        start=(idx == 0), stop=(idx == 17))
                            idx += 1
                    for s in samples:
                        sov = spout[(s, ob)][:].rearrange(
                            "p (h w) -> p h w", h=35, w=35)
                        for ci, (y0, nr) in enumerate([(0, 16), (16, 15)]):
                            pv = ptiles[(s, ci)][:].rearrange(
                                "p (h w) -> p h w", h=nr, w=31)
                            dst = sov[:, 2 + y0:2 + y0 + nr, 2:33]
                            # alternate eviction engines so the trailing
                            # PSUM-free chain is 2-deep per engine, not 4
                            if ci == 0:
                                nc.scalar.activation(
                                    dst, pv[:], RELU,
                                    bias=bs[ob][:, 0:1], scale=1.0)
                            else:
                                nc.vector.tensor_scalar(
                                    dst, pv[:], bs[ob][:, 0:1], 0.0,
                                    mybir.AluOpType.add, mybir.AluOpType.max)

            def xcorr_chunk(g, ob, ci, pool=None, tag="xc", final=False):
                    pool = pool or psx_pool
                    sovs = [spout[(g * 4 + j, ob)][:].rearrange(
                        "p (h w) -> p h w", h=35, w=35) for j in range(4)]
                    for (y0, nr) in [[(0, 16), (16, 15)][ci]]:
                        N = nr * 31
                        px = [pool.tile([128, 512], f32, tag=tag,
                                        name=f"px{g}_{ob}_{y0}_{i}")
                              for i in range(4)]
                        for t in range(25):
                            dy, dx = t // 5, t % 5
                            for i in range(4):
                                for j in range(4):
                                    st = strips[(g * 4 + j, ob)]
                                    nc.tensor.matmul(
                                        px[i][32 * j:32 * j + 32, 0:N],
                                        st[32 * i:32 * i + 32, t * 32:(t + 1) * 32],
                                        sovs[j][32 * i:32 * i + 32,
                                                y0 + dy:y0 + dy + nr, dx:dx + 31],
                                        start=(t == 0), stop=(t == 24),
                                        tile_position=(32 * i, 32 * j))
                        xos = []
                        for i in range(4):
                            xo = xout_pool.tile([128, 496], bf16, tag="xo",
                                                name=f"xo{g}_{ob}_{y0}_{i}")
                            xos.append(xo)
                            if final or i % 2 == 0:
                                # final chunk: all evictions on VectorE so the
                                # ScalarE ring is free to run DMA triggers
                                nc.vector.tensor_copy(xo[:, 0:N], px[i][:, 0:N])
                            else:
                                nc.scalar.copy(xo[:, 0:N], px[i][:, 0:N])
                        # all evictions before all triggers: the scalar FIFO
                        # must not delay copies behind DMA trigger setup
                        for i in range(4):
                            xo = xos[i]
                            dst = out_flat[g * 4:g * 4 + 4,
                                           ob * 128 + 32 * i:ob * 128 + 32 * i + 32,
                                           y0 * 31:y0 * 31 + N]
                            # out-DMA triggered from the (otherwise idle)
                            # GpSimd SWDGE queue so neither SP nor ScalarE
                            # sequencers back up the PE between chunks; the
                            # final chunk splits across both rings to halve
                            # the end-of-kernel drain
                            if final:
                                nc.gpsimd.dma_start(dst[0:2], xo[0:64, 0:N])
                                nc.scalar.dma_start(dst[2:4], xo[64:128, 0:N])
                            else:
                                nc.gpsimd.dma_start(dst, xo[:, 0:N])

            conv_s_group([0])
            conv_s_group([1])
            prefetch_pair(2)
            emit_conv_k()
            emit_strips()
            conv_s_group([2, 3])
            prefetch_pair(3)
            xcorr_chunk(0, 0, 0)
            conv_s_group([4, 5])
            xcorr_chunk(0, 0, 1)
            conv_s_group([6, 7])
            for n, args in enumerate([(0, 1, 0), (0, 1, 1), (1, 0, 0),
                                      (1, 0, 1), (1, 1, 0)]):
                if n % 2 == 0:
                    xcorr_chunk(*args)
                else:
                    xcorr_chunk(*args, pool=psc, tag="conv")
            xcorr_chunk(1, 1, 1, pool=psc, tag="conv", final=True)

    nc.compile()
    return nc


def _host_prep(kernel, search, w_k, g_k, b_k, m_k, v_k, w_s, g_s, b_s, m_s, v_s):
    import ml_dtypes
    BF = ml_dtypes.bfloat16

    def fold(w, g, b, m, v):
        scale = g / np.sqrt(v + EPS)
        return (w * scale[:, None, None, None]).astype(np.float32), \
               (b - m * scale).astype(np.float32)

    wkf, bias_k = fold(w_k, g_k, b_k, m_k, v_k)
    wsf, bias_s = fold(w_s, g_s, b_s, m_s, v_s)

    def packT(w):  # [o, ci, 3, 3] -> [cb, ci, (ob, t, o)] bf16
        arr = w.reshape(2, 128, 2, 128, 9).transpose(2, 3, 0, 4, 1)
        return np.ascontiguousarray(arr).astype(BF).reshape(2, 128, 2304)

    wTk = packT(wkf)
    wTs = packT(wsf)

    M32 = np.zeros((128, 32), dtype=np.float32)
    for p in range(128):
        M32[p, p % 32] = 1.0
    M32 = M32.astype(BF)

    bias_all = np.ascontiguousarray(np.concatenate(
        [bias_k.reshape(2, 128, 1), bias_s.reshape(2, 128, 1)], axis=0))

    in_maps = []
    for core in range(N_CORES):
        kin = kernel[core * SPC:(core + 1) * SPC]
        sin = search[core * SPC:(core + 1) * SPC]

        Xk = np.zeros((2, 128, 9, 200), dtype=np.float32)
        for t in range(9):
            dy, dx = t // 3, t % 3
            p = kin[:, :, dy:dy + 5, dx:dx + 5].reshape(SPC, 2, 128, 25)
            Xk[:, :, t, :] = p.transpose(1, 2, 0, 3).reshape(2, 128, 200)
        Xk = Xk.astype(BF).reshape(2, 128, 1800)

        Xs = np.zeros((SPC, 2, 128, 33, 34), dtype=np.float32)
        Xs[:, :, :, 1:32, 1:32] = sin.reshape(SPC, 2, 128, 31, 31)
        Xs = Xs.astype(BF).reshape(SPC, 2, 128, 33 * 34)

        in_maps.append({
            "wTs": wTs, "wTk": wTk, "xk": Xk,
            "xs": Xs, "bias": bias_all, "m32": M32,
        })
    return in_maps


def kernel(kernel, search, w_k, g_k, b_k, m_k, v_k, w_s, g_s, b_s, m_s, v_s,
           _trace=False):
    global _cached_nc, last_results
    args = [np.ascontiguousarray(np.asarray(x, dtype=np.float32)) for x in
            (kernel, search, w_k, g_k, b_k, m_k, v_k, w_s, g_s, b_s, m_s, v_s)]
    if _cached_nc is None:
        _cached_nc = _build_program()
    nc = _cached_nc
    in_maps = _host_prep(*args)
    res = run_bass_kernel_spmd(nc, in_maps, core_ids=list(range(N_CORES)),
                               trace=_trace)
    last_results = res
    out = np.concatenate([np.asarray(res.results[i]["out"])
                          for i in range(N_CORES)], axis=0)
    return np.ascontiguousarray(out.astype(np.float32))
